# revision 26
# baseline (speedup 1.0000x reference)
"""Trainium2 Bass kernel for nn_DenseAttentionOneHead — collective-free variant.

out_b = X_b (W^T (X_b^T X_b)).  The D (=1024) output columns split into 8
independent 256-column slices (4 per batch): per core,
  S_sl = X_b^T X_b[:, sl]     ([1024, 256], full-batch contraction)
  M_sl = W^T S_sl             ([1024, 256])
  out[:, sl] = X_b M_sl       ([4096, 256])
No inter-core communication.  v3 schedule: the X row-chunk stream owns the
full HBM bandwidth during the S phase — the W load and the X^T column
quarters are gated behind S-phase progress with tiny WAW dependency copies
so the scheduler cannot hoist them.  X^T arrives as 32 independent
[128,1024] tiles so each out-phase quarter starts as soon as it lands.
Output is written in fp16 on the HWDGE rings.  A short warm-up matmul run
brings the PE out of the HAM-throttled clock while the first DMAs fly.
"""

import numpy as np

import concourse.mybir as mybir
import concourse.tile as tile
from concourse import bacc
from concourse.bass_utils import run_bass_kernel_spmd

F32 = mybir.dt.float32
F16 = mybir.dt.float16
P = 128
D = 1024
B = 2
N = 4096
NCORES = 8
GROUP = 4            # cores per batch
SL = D // GROUP      # 256-column slice per core
NO = D // P          # 8
NCH = N // P         # 32 row chunks of the full batch
NQ = 4               # X^T column quarters
NWARM = 20           # PE warm-up matmuls
W_GATE = 30          # release W DMA after this xf chunk has landed
XT_GATE = [31, 31, 31, 31]  # release X^T quarter q after these chunks

_compiled = None


def _build():
    nc = bacc.Bacc(None, target_bir_lowering=False, debug=False, num_devices=NCORES)

    # xf arrives column-rotated per core (its 256 target columns first) and
    # wf row-rotated identically, so the same program computes every slice.
    xf = nc.dram_tensor("xf", [N, D], F16, kind="ExternalInput")
    xt = nc.dram_tensor("xt", [D, N], F16, kind="ExternalInput")
    wf = nc.dram_tensor("wf", [D, D], F16, kind="ExternalInput")
    o_out = nc.dram_tensor("o_out", [N, SL], F16, kind="ExternalOutput")

    with tile.TileContext(nc) as tc:
        with (
            tc.tile_pool(name="big", bufs=1) as big,
            tc.tile_pool(name="xin", bufs=12) as xin,
            tc.tile_pool(name="stage", bufs=4) as stage,
            tc.tile_pool(name="psum", bufs=8, space="PSUM") as psum,
        ):
            Wsb = big.tile([P, NO, D], F16, tag="W")       # W   [e, a], 2MB
            Ssl = big.tile([P, NO, SL], F16, tag="Ssl")    # S_sl [e, d_sl]
            Msl = big.tile([P, NO, SL], F16, tag="Msl")    # M_sl [a, d_sl]
            # X^T as 32 independent column-quarter tiles: Xtq[q][ch] holds
            # X^T[ch*128:(ch+1)*128, q*1024:(q+1)*1024]
            Xtq = [
                [
                    big.tile(
                        [P, D], F16, tag=f"Xt_{q}_{ch}", name=f"Xt_{q}_{ch}"
                    )
                    for ch in range(NO)
                ]
                for q in range(NQ)
            ]
            wsrc = big.tile([P, P], F16, tag="wsrc")       # warm-up operand

            # ---- PE warm-up: dummy matmuls while the first DMAs fly
            nc.vector.memset(wsrc[:], 0.0)
            wacc = psum.tile([P, 512], F32, tag="acc", name="warm")[:, :P]
            for i in range(NWARM):
                nc.tensor.matmul(
                    wacc[:], wsrc[:], wsrc[:], start=True, stop=True
                )
            wout = stage.tile([P, 1], F16, tag="wout")
            nc.vector.tensor_copy(wout[:], wacc[:, :1])    # keep from DCE

            # ---- S_sl = X^T X[:, sl], chunk-outer over the full batch.
            # xf chunks alternate over both HWDGE rings at top priority.
            accs = [
                psum.tile([P, 512], F32, tag="acc", name=f"sacc_{et}")[:, :SL]
                for et in range(NO)
            ]
            xcs = []
            for ch in range(NCH):
                xc = xin.tile([P, D], F16, tag="xc", name=f"xc_{ch}")
                xcs.append(xc)
                if ch == 0:
                    # split the first chunk across both rings so the first
                    # four S matmuls start half a transfer earlier
                    nc.sync.dma_start(xc[:, : D // 2], xf[:P, : D // 2])
                    nc.scalar.dma_start(xc[:, D // 2 :], xf[:P, D // 2 :])
                else:
                    eng = nc.sync if ch % 2 == 0 else nc.scalar
                    eng.dma_start(xc[:], xf[ch * P : (ch + 1) * P, :])
                for et in range(NO):
                    nc.tensor.matmul(
                        accs[et][:],
                        xc[:, et * P : (et + 1) * P],
                        xc[:, :SL],
                        start=(ch == 0),
                        stop=(ch == NCH - 1),
                    )


            # W rides the HWDGE rings between the xf stream and X^T — gated
            # on the last xf chunk's tile so the scheduler cannot hoist it
            # into the S stream (ring FIFO then places it right after xf,
            # landing ~2us before the M phase needs it).
            for pc in range(NO):
                nc.vector.tensor_copy(Wsb[:1, pc, :1], xcs[31][:1, :1])
                eng = nc.sync if pc % 2 == 0 else nc.scalar
                eng.dma_start(Wsb[:, pc, :], wf[pc * P : (pc + 1) * P, :])

            # X^T quarter-major behind staggered gates; quarter q unblocks
            # the out-phase tiles of quarter q alone.
            for q in range(NQ):
                for ch in range(NO):
                    nc.vector.tensor_copy(
                        Xtq[q][ch][:1, :1], xcs[XT_GATE[q]][:1, :1]
                    )
                    eng = nc.sync if ch % 2 == 0 else nc.scalar
                    eng.dma_start(
                        Xtq[q][ch][:],
                        xt[ch * P : (ch + 1) * P, q * D : (q + 1) * D],
                    )

            for et in range(NO):
                nc.vector.tensor_copy(Ssl[:, et, :], accs[et][:])

            # ---- M_sl = W^T S_sl : lhsT = W[e_ch, a_tile], rhs = S_sl[e_ch, :]
            for at in range(NO):
                acc = psum.tile([P, 512], F32, tag="acc", name=f"macc_{at}")[:, :SL]
                for ch in range(NO):
                    nc.tensor.matmul(
                        acc[:],
                        Wsb[:, ch, at * P : (at + 1) * P],
                        Ssl[:, ch, :],
                        start=(ch == 0),
                        stop=(ch == NO - 1),
                    )
                nc.vector.tensor_copy(Msl[:, at, :], acc[:])

            # ---- out[:, sl] = X M_sl : lhsT = X^T quarter blocks, rhs = M_sl
            # Quarters 0-2 run ch-major (8 open accumulators; consecutive
            # matmuls keep lhsT slices within ONE X^T tile and an identical
            # rhs — the S-phase access pattern that sustains ~109ns/MM,
            # vs ~123 when every matmul hops lhsT tiles).  The last quarter
            # stays tile-major so only one copy+write trails the final MM.
            def _emit_out_tile(q, j, acc):
                nt = q * NO + j
                ot = stage.tile([P, SL], F16, tag="ot", name=f"ot_{nt}")
                nc.vector.tensor_copy(ot[:], acc[:])
                if q < 2:
                    nc.gpsimd.dma_start(o_out[nt * P : (nt + 1) * P, :], ot[:])
                else:
                    weng = nc.sync if nt % 2 == 0 else nc.scalar
                    weng.dma_start(o_out[nt * P : (nt + 1) * P, :], ot[:])

            for q in range(NQ - 1):
                oaccs = [
                    psum.tile([P, 512], F32, tag="acc", name=f"oacc_{q}_{j}")[
                        :, :SL
                    ]
                    for j in range(NO)
                ]
                for ch in range(NO):
                    for j in range(NO):
                        nc.tensor.matmul(
                            oaccs[j][:],
                            Xtq[q][ch][:, j * P : (j + 1) * P],
                            Msl[:, ch, :],
                            start=(ch == 0),
                            stop=(ch == NO - 1),
                        )
                for j in range(NO):
                    _emit_out_tile(q, j, oaccs[j])

            for j in range(NO):
                q = NQ - 1
                acc = psum.tile(
                    [P, 512], F32, tag="acc", name=f"oacc_{q}_{j}"
                )[:, :SL]
                for ch in range(NO):
                    nc.tensor.matmul(
                        acc[:],
                        Xtq[q][ch][:, j * P : (j + 1) * P],
                        Msl[:, ch, :],
                        start=(ch == 0),
                        stop=(ch == NO - 1),
                    )
                _emit_out_tile(q, j, acc)


    nc.finalize()
    return nc


def _get_compiled():
    global _compiled
    if _compiled is None:
        _compiled = _build()
    return _compiled


def kernel(hidden_states, queries, _trace=False, _trace_cores=None):
    x = np.ascontiguousarray(np.asarray(hidden_states, dtype=np.float32))
    w = np.ascontiguousarray(np.asarray(queries, dtype=np.float32))
    assert x.shape == (B, N, D) and w.shape == (D, D)

    nc = _get_compiled()
    w16 = w.astype(np.float16)
    x16 = [x[b].astype(np.float16) for b in range(B)]
    xt16 = [np.ascontiguousarray(x16[b].T) for b in range(B)]
    in_maps = []
    for c in range(NCORES):
        b, s = c // GROUP, c % GROUP
        in_maps.append(
            {
                "xf": np.ascontiguousarray(np.roll(x16[b], -s * SL, axis=1)),
                "xt": xt16[b],
                "wf": np.ascontiguousarray(np.roll(w16, -s * SL, axis=0)),
            }
        )

    res = run_bass_kernel_spmd(
        nc,
        in_maps,
        core_ids=list(range(NCORES)),
        trace=_trace,
        trace_cores=_trace_cores,
    )

    out = np.empty((B, N, D), dtype=np.float32)
    for c in range(NCORES):
        b, s = c // GROUP, c % GROUP
        out[b, :, s * SL : (s + 1) * SL] = res.results[c]["o_out"].astype(np.float32)

    if _trace:
        kernel.last_result = res
    return out
